# revision 8
# baseline (speedup 1.0000x reference)
"""Chamfer 3D loss kernel for Trainium2 (8 NeuronCores).

Strategy
--------
Shard over B (data parallel): each of the 8 cores handles one batch item.

Per core, for p [3,4096] and g [3,4096] we need the bidirectional nearest
neighbour distances of the 4096x4096 pair matrix.  We build the *negated*
squared distance matrix
    negdist[m,n] = 2 p_m . g_n - |p_m|^2 - |g_n|^2
with a single K=24 bf16 matmul per tile: every fp32 operand is split into
a sum of bf16 terms (3-way mantissa split) and the rank-1 correction rows
(-|p|^2 and -|g|^2 against ones) are stacked along the contraction axis.
bf16 matmuls run at 1 cycle/row on the PE (vs 4 for fp32) and the fp32
PSUM accumulation keeps ~1e-7 relative accuracy on the final loss.

The 16.7M-element matrix is consumed twice.  ScalarE cast-copies each PSUM
chunk to fp16 in SBUF (~2x perf mode, ~64us total — not the bottleneck),
then the two reductions are split between VectorE and the DMA fabric to
balance their rooflines (DVE 0.96GHz / TT f16 2x_1P vs ~358GB/s HBM per
core):
  * bwd (min over m per n): running elementwise fp16 max on VectorE
    (2194ns/chunk — irreducible on DVE, and no other engine can do
    two-tensor max: walrus rejects Pool/GPSIMD tensor_tensor max and DMA
    CCE max; tensor_tensor_reduce NEFF is rejected by the runtime).
  * fwd (min over n per m): only cols [0:1024) are folded on VectorE
    (FOLD=512 outputs, ~327ns); cols [1024:4096) are DMA'd out *raw* as
    fp16 and reduced on the host.  This converts ~800ns/chunk of VectorE
    time into DMA bytes; at ~914KB/chunk the DMA fabric runs just at its
    roofline, matching VectorE's ~2.5us/chunk.
Engine totals per chunk: DVE ~2520ns, DMA ~2560ns, ACT ~2000ns, PE ~1730ns.

Final sqrt / mean runs on host in float64 (ScalarE sqrt has a loose ULP
budget and the data is only ~1MB per core).
"""

import sys

sys.path.insert(0, "/opt/trn_rl_repo")

import numpy as np
import ml_dtypes

B, C, M, N = 8, 3, 4096, 4096
KROWS = 24
NCORES = 8
EPS = 1e-8
XACC = 1632  # cols [0:XACC) reduced on DVE (acc + 3 folds); rest DMA'd raw
NFOLD = 3  # fold depth on the acc region: XACC -> XACC/2**NFOLD fwd outputs

_prog = None


def build_program(reps=None):
    """Build the per-core program.  reps=None -> real kernel (external
    outputs); reps=R -> body wrapped in a tc.For_i hardware loop with
    internal-DRAM outputs, for loop-delta device timing."""
    import concourse.bass as bass
    import concourse.mybir as mybir
    from concourse import bacc, tile

    f32 = mybir.dt.float32
    f16 = mybir.dt.float16
    bf16 = mybir.dt.bfloat16
    OP = mybir.AluOpType
    RAW = N - XACC
    FOUT = XACC >> NFOLD

    nc = bacc.Bacc("TRN2", target_bir_lowering=False, debug=False)

    timing = reps is not None
    kind = dict(kind="ExternalOutput") if not timing else {}
    a_d = nc.dram_tensor("a", [KROWS, M], bf16, kind="ExternalInput")
    b_d = nc.dram_tensor("b", [KROWS, N], bf16, kind="ExternalInput")
    fold_d = nc.dram_tensor("foldout", [32, 128, FOUT], f16, **kind)
    raw_d = nc.dram_tensor("rawout", [32, 128, RAW], f16, **kind)
    acc_d = nc.dram_tensor("acc", [128, XACC], f16, **kind)
    if timing:
        y_d = nc.dram_tensor("y", [128, 2], f32, kind="ExternalOutput")

    with tile.TileContext(nc) as tc:
        with (
            tc.tile_pool(name="const", bufs=1) as cpool,
            tc.tile_pool(name="stage", bufs=4) as spool,
            tc.tile_pool(name="psum", bufs=2, space=bass.MemorySpace.PSUM) as ppool,
        ):
            a_s = cpool.tile([KROWS, M], bf16)
            b_s = cpool.tile([KROWS, N], bf16)
            nc.sync.dma_start(a_s[:], a_d.ap())
            nc.sync.dma_start(b_s[:], b_d.ap())

            acc = cpool.tile([128, XACC], f16)
            nc.vector.memset(acc[:], -60000.0)
            if timing:
                yt = cpool.tile([128, 2], f32)
                nc.vector.memset(yt[:], 0.0)

            import contextlib

            loop = tc.For_i(0, reps, 1) if timing else contextlib.nullcontext()
            with loop:
                for mi in range(32):
                    ct = spool.tile([128, N], f16)
                    for half in range(2):
                        pt = ppool.tile([128, 2048], f32)
                        for j in range(4):
                            nj = half * 4 + j
                            nc.tensor.matmul(
                                pt[:, j * 512 : (j + 1) * 512],
                                a_s[:, mi * 128 : (mi + 1) * 128],
                                b_s[:, nj * 512 : (nj + 1) * 512],
                            )
                        nc.scalar.copy(
                            ct[:, half * 2048 : (half + 1) * 2048], pt[:]
                        )
                    nc.sync.dma_start(raw_d.ap()[mi], ct[:, XACC:])
                    prev = ct
                    w = XACC
                    for _ in range(NFOLD):
                        w //= 2
                        t = spool.tile([128, w], f16)
                        nc.vector.tensor_tensor(
                            t[:], prev[:, :w], prev[:, w : 2 * w], op=OP.max
                        )
                        prev = t
                    nc.sync.dma_start(fold_d.ap()[mi], prev[:])
                    nc.vector.tensor_tensor(
                        acc[:], acc[:], ct[:, :XACC], op=OP.max
                    )
            nc.sync.dma_start(acc_d.ap(), acc[:])
            if timing:
                nc.sync.dma_start(y_d.ap(), yt[:])

    nc.compile()
    return nc


def _get_program():
    global _prog
    if _prog is None:
        _prog = build_program()
    return _prog


def _split3(x64):
    bf = ml_dtypes.bfloat16
    x1 = x64.astype(bf)
    r = x64 - x1.astype(np.float64)
    x2 = r.astype(bf)
    x3 = (r - x2.astype(np.float64)).astype(bf)
    return x1, x2, x3


def _prep_one(p, g):
    """p, g: [3, 4096] float32 -> (A, B) [24, 4096] bf16 each."""
    bf = ml_dtypes.bfloat16
    p = p.astype(np.float64)
    g = g.astype(np.float64)
    u1, u2, u3 = _split3(2.0 * p)
    b1, b2, b3 = _split3(g)
    s1, s2, s3 = _split3(-(p * p).sum(0))
    t1, t2, t3 = _split3(-(g * g).sum(0))
    ones = np.ones(p.shape[1], dtype=bf)
    arows, brows = [], []
    for c in range(3):
        for i, j in ((0, 0), (0, 1), (0, 2), (1, 0), (1, 1), (2, 0)):
            arows.append((u1, u2, u3)[i][c])
            brows.append((b1, b2, b3)[j][c])
    for s in (s1, s2, s3):
        arows.append(s)
        brows.append(ones)
    for t in (t1, t2, t3):
        arows.append(ones)
        brows.append(t)
    return np.stack(arows).astype(bf), np.stack(brows).astype(bf)


def _prep_in_maps(predict_pc, gt_pc):
    in_maps = []
    for b in range(B):
        A, Bm = _prep_one(predict_pc[b, :3], gt_pc[b, :3])
        in_maps.append({"a": A, "b": Bm})
    return in_maps


def run_on_cores(in_maps, trace=False, tmpdir=None):
    from concourse.bass_utils import run_bass_kernel_spmd

    nc = _get_program()
    return run_bass_kernel_spmd(
        nc, in_maps, list(range(NCORES)), trace=trace, tmpdir=tmpdir
    )


def _postprocess(results):
    total = 0.0
    for b in range(B):
        r = results[b]
        # fwd: per m = mi*128 + lane, min over n = max over negdist
        fp = r["foldout"].astype(np.float32)  # [32, 128, FOUT] covers n [0:XACC)
        rp = r["rawout"].astype(np.float32)  # [32, 128, RAW] covers n [XACC:N)
        mx = np.maximum(fp.max(axis=2), rp.max(axis=2))  # [32, 128]
        d2f = -mx.reshape(M).astype(np.float64)
        # bwd: per n, max over all m. cols [0:XACC) from acc (max over lanes);
        # cols [XACC:N) from the raw dump (max over chunk x lane).
        bl = r["acc"].max(axis=0)  # [XACC]
        br = rp.max(axis=(0, 1))  # [RAW]
        d2b = -np.concatenate([bl, br]).astype(np.float64)
        total += np.sqrt(np.maximum(d2f, 0.0) + EPS).sum()
        total += np.sqrt(np.maximum(d2b, 0.0) + EPS).sum()
    return np.float32(total / (B * M))


def kernel(predict_pc, gt_pc):
    predict_pc = np.asarray(predict_pc, dtype=np.float32)
    gt_pc = np.asarray(gt_pc, dtype=np.float32)
    in_maps = _prep_in_maps(predict_pc, gt_pc)
    res = run_on_cores(in_maps)
    return _postprocess(res.results)


# revision 10
# speedup vs baseline: 1.5336x; 1.5336x over previous
"""Chamfer 3D loss kernel for Trainium2 (8 NeuronCores).

Strategy
--------
Shard over B (data parallel): each of the 8 cores handles one batch item.

Per core, for p [3,4096] and g [3,4096] we need the bidirectional nearest
neighbour distances of the 4096x4096 pair matrix.  We build the *negated*
squared distance matrix
    negdist[m,n] = 2 p_m . g_n - |p_m|^2 - |g_n|^2
with a single K=24 bf16 matmul per tile: every fp32 operand is split into
a sum of bf16 terms (3-way mantissa split) and the rank-1 correction rows
(-|p|^2 and -|g|^2 against ones) are stacked along the contraction axis.
bf16 matmuls run at 1 cycle/row on the PE (vs 4 for fp32) and the fp32
PSUM accumulation keeps ~1e-7 relative accuracy on the final loss.

The 16.7M-element matrix is consumed twice.  ScalarE cast-copies each PSUM
chunk to fp16 in SBUF (not the bottleneck), then the two reductions are
split between VectorE and the DMA fabric so both run at their rooflines
(DVE 0.96GHz, TT f16 2x_1P = 2 el/cyc/lane; DMA ~374GB/s HBM per core):
  * cols [0:XACC): bwd handled by a running elementwise fp16 max on
    VectorE (acc, ~860ns/chunk); fwd handled by 3 levels of pairwise
    TT-max folds (1536->192, ~880ns/chunk), whose 192-el result is DMA'd
    out and finished on host.  (No other engine can do two-tensor max:
    walrus rejects Pool/GPSIMD tensor_tensor max and DMA CCE max;
    tensor_tensor_reduce's NEFF is rejected by the runtime; fp8 raw
    output fails the 2e-2 tolerance at rel_err ~1.7e-2.)
  * cols [XACC:4096): DMA'd out *raw* as fp16 (~655KB/chunk); the host
    computes BOTH the fwd and bwd contributions of these columns, so each
    raw byte relieves VectorE of both reductions.
Per-chunk engine totals: DVE ~1740ns, DMA ~704KB = ~1880ns (binding),
ACT <=1880ns, PE ~1730ns -> measured 60.2us steady-state (1881ns/chunk),
vs 106us for the all-on-DVE structure and ~55us PE roofline.

Final sqrt / mean runs on host in float64 (ScalarE sqrt has a loose ULP
budget and the data is only ~1MB per core).
"""

import sys

sys.path.insert(0, "/opt/trn_rl_repo")

import numpy as np
import ml_dtypes

B, C, M, N = 8, 3, 4096, 4096
KROWS = 24
NCORES = 8
EPS = 1e-8
XACC = 1536  # cols [0:XACC) reduced on DVE (acc + 3 folds); rest DMA'd raw
# NOTE: XACC=1632 (the model-optimal balance point) measures 2.6x SLOWER
# (156us vs 60us) — some size-triggered slow path (likely a DVE perf-mode
# fallback / DMA pattern pathology for non-512-multiple widths).  1536 is
# measured-optimal; keep XACC a multiple of 512.
NFOLD = 3  # fold depth on the acc region: XACC -> XACC/2**NFOLD fwd outputs

_prog = None


def build_program(reps=None):
    """Build the per-core program.  reps=None -> real kernel (external
    outputs); reps=R -> body wrapped in a tc.For_i hardware loop with
    internal-DRAM outputs, for loop-delta device timing."""
    import concourse.bass as bass
    import concourse.mybir as mybir
    from concourse import bacc, tile

    f32 = mybir.dt.float32
    f16 = mybir.dt.float16
    bf16 = mybir.dt.bfloat16
    OP = mybir.AluOpType
    RAW = N - XACC
    FOUT = XACC >> NFOLD

    nc = bacc.Bacc("TRN2", target_bir_lowering=False, debug=False)

    timing = reps is not None
    kind = dict(kind="ExternalOutput") if not timing else {}
    a_d = nc.dram_tensor("a", [KROWS, M], bf16, kind="ExternalInput")
    b_d = nc.dram_tensor("b", [KROWS, N], bf16, kind="ExternalInput")
    fold_d = nc.dram_tensor("foldout", [32, 128, FOUT], f16, **kind)
    raw_d = nc.dram_tensor("rawout", [32, 128, RAW], f16, **kind)
    acc_d = nc.dram_tensor("acc", [128, XACC], f16, **kind)
    if timing:
        y_d = nc.dram_tensor("y", [128, 2], f32, kind="ExternalOutput")

    with tile.TileContext(nc) as tc:
        with (
            tc.tile_pool(name="const", bufs=1) as cpool,
            tc.tile_pool(name="stage", bufs=4) as spool,
            tc.tile_pool(name="psum", bufs=2, space=bass.MemorySpace.PSUM) as ppool,
        ):
            a_s = cpool.tile([KROWS, M], bf16)
            b_s = cpool.tile([KROWS, N], bf16)
            nc.sync.dma_start(a_s[:], a_d.ap())
            nc.sync.dma_start(b_s[:], b_d.ap())

            acc = cpool.tile([128, XACC], f16)
            nc.vector.memset(acc[:], -60000.0)
            if timing:
                yt = cpool.tile([128, 2], f32)
                nc.vector.memset(yt[:], 0.0)

            import contextlib

            loop = tc.For_i(0, reps, 1) if timing else contextlib.nullcontext()
            with loop:
                for mi in range(32):
                    ct = spool.tile([128, N], f16)
                    for half in range(2):
                        pt = ppool.tile([128, 2048], f32)
                        for j in range(4):
                            nj = half * 4 + j
                            nc.tensor.matmul(
                                pt[:, j * 512 : (j + 1) * 512],
                                a_s[:, mi * 128 : (mi + 1) * 128],
                                b_s[:, nj * 512 : (nj + 1) * 512],
                            )
                        nc.scalar.copy(
                            ct[:, half * 2048 : (half + 1) * 2048], pt[:]
                        )
                    nc.sync.dma_start(raw_d.ap()[mi], ct[:, XACC:])
                    prev = ct
                    w = XACC
                    for _ in range(NFOLD):
                        w //= 2
                        t = spool.tile([128, w], f16)
                        nc.vector.tensor_tensor(
                            t[:], prev[:, :w], prev[:, w : 2 * w], op=OP.max
                        )
                        prev = t
                    nc.sync.dma_start(fold_d.ap()[mi], prev[:])
                    nc.vector.tensor_tensor(
                        acc[:], acc[:], ct[:, :XACC], op=OP.max
                    )
            nc.sync.dma_start(acc_d.ap(), acc[:])
            if timing:
                nc.sync.dma_start(y_d.ap(), yt[:])

    nc.compile()
    return nc


def _get_program():
    global _prog
    if _prog is None:
        _prog = build_program()
    return _prog


def _split3(x64):
    bf = ml_dtypes.bfloat16
    x1 = x64.astype(bf)
    r = x64 - x1.astype(np.float64)
    x2 = r.astype(bf)
    x3 = (r - x2.astype(np.float64)).astype(bf)
    return x1, x2, x3


def _prep_one(p, g):
    """p, g: [3, 4096] float32 -> (A, B) [24, 4096] bf16 each."""
    bf = ml_dtypes.bfloat16
    p = p.astype(np.float64)
    g = g.astype(np.float64)
    u1, u2, u3 = _split3(2.0 * p)
    b1, b2, b3 = _split3(g)
    s1, s2, s3 = _split3(-(p * p).sum(0))
    t1, t2, t3 = _split3(-(g * g).sum(0))
    ones = np.ones(p.shape[1], dtype=bf)
    arows, brows = [], []
    for c in range(3):
        for i, j in ((0, 0), (0, 1), (0, 2), (1, 0), (1, 1), (2, 0)):
            arows.append((u1, u2, u3)[i][c])
            brows.append((b1, b2, b3)[j][c])
    for s in (s1, s2, s3):
        arows.append(s)
        brows.append(ones)
    for t in (t1, t2, t3):
        arows.append(ones)
        brows.append(t)
    return np.stack(arows).astype(bf), np.stack(brows).astype(bf)


def _prep_in_maps(predict_pc, gt_pc):
    in_maps = []
    for b in range(B):
        A, Bm = _prep_one(predict_pc[b, :3], gt_pc[b, :3])
        in_maps.append({"a": A, "b": Bm})
    return in_maps


def run_on_cores(in_maps, trace=False, tmpdir=None):
    from concourse.bass_utils import run_bass_kernel_spmd

    nc = _get_program()
    return run_bass_kernel_spmd(
        nc, in_maps, list(range(NCORES)), trace=trace, tmpdir=tmpdir
    )


def _postprocess(results):
    total = 0.0
    for b in range(B):
        r = results[b]
        # fwd: per m = mi*128 + lane, min over n = max over negdist
        fp = r["foldout"].astype(np.float32)  # [32, 128, FOUT] covers n [0:XACC)
        rp = r["rawout"].astype(np.float32)  # [32, 128, RAW] covers n [XACC:N)
        mx = np.maximum(fp.max(axis=2), rp.max(axis=2))  # [32, 128]
        d2f = -mx.reshape(M).astype(np.float64)
        # bwd: per n, max over all m. cols [0:XACC) from acc (max over lanes);
        # cols [XACC:N) from the raw dump (max over chunk x lane).
        bl = r["acc"].max(axis=0)  # [XACC]
        br = rp.max(axis=(0, 1))  # [RAW]
        d2b = -np.concatenate([bl, br]).astype(np.float64)
        total += np.sqrt(np.maximum(d2f, 0.0) + EPS).sum()
        total += np.sqrt(np.maximum(d2b, 0.0) + EPS).sum()
    return np.float32(total / (B * M))


def kernel(predict_pc, gt_pc):
    predict_pc = np.asarray(predict_pc, dtype=np.float32)
    gt_pc = np.asarray(gt_pc, dtype=np.float32)
    in_maps = _prep_in_maps(predict_pc, gt_pc)
    res = run_on_cores(in_maps)
    return _postprocess(res.results)
